# revision 9
# baseline (speedup 1.0000x reference)
"""Trainium2 Bass kernel for the DKT (graph-based knowledge tracing) model.

Sharding across the 8 NeuronCores:
  - GCN phase: row-shard of the three [5000,5000] adjacency matmuls (625 rows
    per core), with AllGathers of the small [5000,EMB] intermediates.
  - x@ques / GRU / logit heads: data-parallel over batch (8 sequences/core).

Everything large is bf16 (rel-err budget 2e-2; measured ~1e-3): halves HBM
traffic (the kernel is memory-bound) and enables fast-weight-load on the PE
for the GRU's tiny recurrence matmuls. PSUM accumulation stays fp32.

Layouts are host-packed so every big DMA is contiguous per partition:
  - at_g   [NQ, 640]        A^T column-shard, padded 625->640
  - z1p_g  [125, 40*128]    layer-1 GCN activations, contraction-chunk packed
  - xtp    [125, 40*1600]   x^T batch-shard, contraction-chunk packed
GRU per step t uses one PSUM tile [128,48] = [r0 r1 z0 z1 n0 n1] built by one
identity-matmul (folds the precomputed input projections + bhh_n bias) plus
six bf16 Whh matmuls; gates then take 2 scalar + 5 vector ops per step for
both GRUs together.
"""

import numpy as np
import ml_dtypes

Q = 2500
NQ = 5000
EMB = 128
H = 128
B = 64
L = 200
NCORES = 8
SHARD = NQ // NCORES          # 625 adjacency rows per core
KC = 125                      # contraction chunk (partition dim)
NK = NQ // KC                 # 40 chunks
BLOC = B // NCORES            # 8 sequences per core
BLC = L * BLOC                # 1600 (bl index = t*8 + b, t-major)
SHARD_P = 640                 # shard padded to even halves
NH = [(0, 320), (320, 320)]   # padded-shard column halves
AC = 10                       # A-rows chunks per DMA (10*125 rows = 1.6MB bf16)
XC = 4                        # xt k-chunks per DMA (1.6MB bf16)
XNT = [(i * 400, 400) for i in range(4)]               # x-stage N tiles
HNT = [(0, 512), (512, 512), (1024, 512), (1536, 512), (2048, 452)]
XPW = 64                      # xp columns per step (see layout below)

_BUILT = None
LAST = None


def _build(debug=False):
    import concourse.bass as bass  # noqa: F401
    import concourse.tile as tile
    from concourse import bacc, mybir
    from concourse.masks import make_identity
    from contextlib import ExitStack

    f32 = mybir.dt.float32
    bf16 = mybir.dt.bfloat16
    AFT = mybir.ActivationFunctionType
    ALU = mybir.AluOpType

    nc = bacc.Bacc("TRN2", target_bir_lowering=False, debug=False,
                   num_devices=NCORES)

    def din(name, shape, dt=bf16):
        return nc.dram_tensor(name, shape, dt, kind="ExternalInput").ap()

    def dout(name, shape, dt=bf16):
        return nc.dram_tensor(name, shape, dt, kind="ExternalOutput").ap()

    # --- inputs (per-core unless noted) ---
    at = {g: din(f"at_{g}", [NQ, SHARD_P]) for g in ("hg", "g1", "g2")}
    xt = din("xt", [KC, NK * BLC])
    z1 = {g: din(f"z1_{g}", [KC, NK * EMB]) for g in ("hg", "g1", "g2")}
    e2s = {"hg": EMB, "g1": EMB // 2, "g2": EMB // 2}
    w2 = {g: din(f"w2_{g}", [EMB, e2s[g]]) for g in ("hg", "g1", "g2")}
    b2 = {g: din(f"b2_{g}", [1, e2s[g]]) for g in ("hg", "g1", "g2")}
    wihT = [din("wihT1", [EMB, 3 * H]), din("wihT2", [EMB, 3 * H])]
    whhT = [din("whhT1", [EMB, 3 * H]), din("whhT2", [EMB, 3 * H])]
    projb = [din("projb1", [EMB, 3], f32), din("projb2", [EMB, 3], f32)]
    bhhn = [din("bhhn1", [EMB, 1], f32), din("bhhn2", [EMB, 1], f32)]
    w1wT = din("w1wT", [EMB, EMB])
    w2wT = din("w2wT", [EMB, EMB])
    wb = din("wb", [EMB, 1], f32)
    fccwT = din("fccwT", [EMB, Q])
    fctwT = din("fctwT", [EMB, Q])
    fcewT = din("fcewT", [2 * EMB, Q])

    out_c = dout("out_c", [L, BLOC, Q])
    out_t = dout("out_t", [L, BLOC, Q])
    out_e = dout("out_e", [L, BLOC, Q])

    with tile.TileContext(nc) as tc, ExitStack() as ctx:
        const = ctx.enter_context(tc.tile_pool(name="const", bufs=1))
        dram = ctx.enter_context(tc.tile_pool(name="dram", bufs=1, space="DRAM"))

        ident = const.tile([128, 128], f32, name="ident")
        make_identity(nc, ident[:])
        ident_b = const.tile([128, 128], bf16, name="ident_b")
        nc.vector.tensor_copy(ident_b[:], ident[:])
        ones_f = const.tile([1, 128], f32, name="ones_f")
        nc.gpsimd.memset(ones_f[:], 1.0)
        ones = const.tile([1, 128], bf16, name="ones")
        nc.vector.tensor_copy(ones[:], ones_f[:])

        # DRAM bounce buffers for the AllGathers
        zb = {"hg": dram.tile([SHARD, EMB], bf16, name="zb_hg"),
              "pr": dram.tile([SHARD, EMB], bf16, name="zb_pr")}
        zf = {"hg": dram.tile([NQ, EMB], bf16, name="zf_hg", addr_space="Shared"),
              "pr": dram.tile([NQ, EMB], bf16, name="zf_pr", addr_space="Shared")}
        qb = {"hg": dram.tile([SHARD, EMB], bf16, name="qb_hg"),
              "pr": dram.tile([SHARD, EMB], bf16, name="qb_pr")}
        qf = {"hg": dram.tile([NQ, EMB], bf16, name="qf_hg", addr_space="Shared"),
              "pr": dram.tile([NQ, EMB], bf16, name="qf_pr", addr_space="Shared")}
        RG = [list(range(NCORES))]

        def allgather(inb, outb):
            nc.gpsimd.collective_compute(
                "AllGather", ALU.bypass, replica_groups=RG,
                ins=[inb.opt()], outs=[outb.opt()])

        def rearr_kpe(ap, e):
            return ap.rearrange("(k p) e -> p k e", p=KC)

        # Three independent DMA queues on TRN2: gpsimd (SWDGE, sprays all 16
        # SDMA engines) + the sync/scalar HWDGE queues (both drive engines
        # 0-4 only). Weight the round-robin heavily toward gpsimd so engines
        # 0-4 don't saturate, while the HWDGE queues still break the
        # one-DMA-at-a-time serialization of the single SWDGE queue.
        _dmaq = [nc.gpsimd, nc.gpsimd, nc.gpsimd, nc.gpsimd, nc.sync,
                 nc.gpsimd, nc.gpsimd, nc.gpsimd, nc.gpsimd, nc.scalar]
        _dma_ctr = [0]

        def big_dma(out, in_):
            q = _dmaq[_dma_ctr[0] % len(_dmaq)]
            _dma_ctr[0] += 1
            q.dma_start(out, in_)

        # ================= GCN phase =================
        sbQ = ctx.enter_context(tc.tile_pool(name="sbQ", bufs=1))
        qh_sb = sbQ.tile([KC, NK * EMB], bf16, name="qh_sb")
        qd_sb = sbQ.tile([KC, NK * EMB], bf16, name="qd_sb")
        with tc.tile_pool(name="sbG", bufs=1) as sbG, \
             tc.tile_pool(name="astream", bufs=3) as astream, \
             tc.tile_pool(name="psA", bufs=4, space="PSUM") as psA, \
             tc.tile_pool(name="psW", bufs=2, space="PSUM") as psW, \
             tc.tile_pool(name="psT", bufs=2, space="PSUM") as psT:

            z1sb, hT, w2sb, b2sb = {}, {}, {}, {}
            zstag, qstag, z2f = {}, {}, {}

            def gcn_stage1(g):
                z1sb[g] = sbG.tile([KC, NK * EMB], bf16, name=f"z1sb_{g}",
                                   tag="z1sb", bufs=2)
                nc.gpsimd.dma_start(z1sb[g][:], z1[g][:])
                w2sb[g] = sbG.tile([EMB, e2s[g]], bf16, name=f"w2sb_{g}")
                nc.sync.dma_start(w2sb[g][:], w2[g][:])
                b2sb[g] = sbG.tile([1, e2s[g]], bf16, name=f"b2sb_{g}")
                nc.sync.dma_start(b2sb[g][:], b2[g][:])
                hT[g] = sbG.tile([EMB, SHARD_P], bf16, name=f"hT_{g}",
                                 tag="hT", bufs=2)

                ps = [psA.tile([EMB, 512], f32, name=f"ps1_{g}{i}", tag="psA")
                      for i in range(2)]
                for k4 in range(NK // AC):
                    a_t = astream.tile([KC, AC * SHARD_P], bf16, name="a_t",
                                       tag="a")
                    big_dma(a_t.rearrange("p (c s) -> p c s", c=AC),
                            at[g][AC * k4 * KC:(AC * k4 + AC) * KC, :]
                            .rearrange("(c p) s -> p c s", c=AC))
                    for c in range(AC):
                        k = AC * k4 + c
                        for i, (off, nh) in enumerate(NH):
                            nc.tensor.matmul(
                                ps[i][:, :nh],
                                z1sb[g][:, k * EMB:(k + 1) * EMB],
                                a_t[:, c * SHARD_P + off:c * SHARD_P + off + nh],
                                start=(k == 0), stop=(k == NK - 1))
                for i, (off, nh) in enumerate(NH):
                    nc.scalar.activation(hT[g][:, off:off + nh],
                                         ps[i][:EMB, :nh], AFT.Relu)

            def gcn_stage2w(g, grp, coloff):
                # Z2 = h @ W2 + b2 (natural layout, into the AG input staging)
                e2 = e2s[g]
                if grp not in zstag:
                    zstag[grp] = sbG.tile([KC, 5 * EMB], bf16,
                                          name=f"zstag_{grp}")
                for c in range(5):
                    ps = psW.tile([KC, EMB], f32, name="psW", tag="psW")
                    nc.tensor.matmul(ps[:, :e2], hT[g][:, c * KC:(c + 1) * KC],
                                     w2sb[g][:], start=True, stop=False)
                    nc.tensor.matmul(ps[:, :e2], ones[:, :KC], b2sb[g][:],
                                     start=False, stop=True)
                    nc.vector.tensor_copy(
                        zstag[grp][:, c * EMB + coloff: c * EMB + coloff + e2],
                        ps[:, :e2])

            def ag_z(grp):
                nc.sync.dma_start(
                    zb[grp].rearrange("(c p) e -> p c e", p=KC),
                    zstag[grp].rearrange("p (c e) -> p c e", c=5))
                allgather(zb[grp], zf[grp])
                z2f[grp] = sbG.tile([KC, NK * EMB], bf16, name=f"z2f_{grp}",
                                    tag="z2f", bufs=2)
                nc.gpsimd.dma_start(
                    z2f[grp].rearrange("p (k e) -> p k e", k=NK),
                    rearr_kpe(zf[grp], EMB))

            def gcn_stage2a(g, grp, coloff):
                e2 = e2s[g]
                o2T = sbG.tile([e2, SHARD_P], f32, name=f"o2T_{g}", tag="o2T",
                               bufs=2)
                ps = [psA.tile([EMB, 512], f32, name=f"ps2_{g}{i}", tag="psA")
                      for i in range(2)]
                for k4 in range(NK // AC):
                    a_t = astream.tile([KC, AC * SHARD_P], bf16, name="a_t2",
                                       tag="a")
                    big_dma(a_t.rearrange("p (c s) -> p c s", c=AC),
                            at[g][AC * k4 * KC:(AC * k4 + AC) * KC, :]
                            .rearrange("(c p) s -> p c s", c=AC))
                    for c in range(AC):
                        k = AC * k4 + c
                        for i, (off, nh) in enumerate(NH):
                            nc.tensor.matmul(
                                ps[i][:e2, :nh],
                                z2f[grp][:, k * EMB + coloff: k * EMB + coloff + e2],
                                a_t[:, c * SHARD_P + off:c * SHARD_P + off + nh],
                                start=(k == 0), stop=(k == NK - 1))
                for i, (off, nh) in enumerate(NH):
                    nc.vector.tensor_copy(o2T[:, off:off + nh], ps[i][:e2, :nh])
                # transpose to natural layout; stage for the output AllGather
                if grp not in qstag:
                    qstag[grp] = sbG.tile([KC, 5 * EMB], bf16,
                                          name=f"qstag_{grp}")
                # ques_d = concat([ques_in(g2), ques_out(g1)]): g2 -> cols
                # 0:64, g1 -> cols 64:128 of each block; hg -> full block.
                qoff = {"hg": 0, "g1": 64, "g2": 0}[g]
                for c in range(5):
                    pst = psT.tile([KC, EMB], f32, name="psT", tag="psT")
                    nc.tensor.transpose(pst[:, :e2],
                                        o2T[:, c * KC:(c + 1) * KC],
                                        ident[:e2, :e2])
                    nc.vector.tensor_copy(
                        qstag[grp][:, c * EMB + qoff: c * EMB + qoff + e2],
                        pst[:, :e2])

            def ag_q(grp):
                nc.sync.dma_start(
                    qb[grp].rearrange("(c p) e -> p c e", p=KC),
                    qstag[grp].rearrange("p (c e) -> p c e", c=5))
                allgather(qb[grp], qf[grp])

            gcn_stage1("hg")
            gcn_stage2w("hg", "hg", 0)
            ag_z("hg")
            gcn_stage1("g1")
            gcn_stage2w("g1", "pr", 0)
            gcn_stage1("g2")
            gcn_stage2w("g2", "pr", 64)
            ag_z("pr")
            gcn_stage2a("hg", "hg", 0)
            ag_q("hg")
            nc.gpsimd.dma_start(qh_sb.rearrange("p (k e) -> p k e", k=NK),
                                rearr_kpe(qf["hg"], EMB))
            gcn_stage2a("g1", "pr", 0)
            gcn_stage2a("g2", "pr", 64)
            ag_q("pr")
            nc.gpsimd.dma_start(qd_sb.rearrange("p (k e) -> p k e", k=NK),
                                rearr_kpe(qf["pr"], EMB))

        # ================= x @ ques phase =================
        # xp layout per step t (XPW=64 cols):
        #   0:8 xr_u0 | 8:16 xr_u1 | 16:24 xz_u0 | 24:32 xz_u1
        #   32:40 bhhn_u0 | 40:48 bhhn_u1 | 48:56 xn_u0 | 56:64 xn_u1
        sbP = ctx.enter_context(tc.tile_pool(name="sbP", bufs=1))
        xp = sbP.tile([EMB, L * XPW], bf16, name="xp")
        xp_t = xp.rearrange("p (t c) -> p t c", c=XPW)

        with tc.tile_pool(name="sbX", bufs=1) as sbX, \
             tc.tile_pool(name="xstream", bufs=3) as xstream:
            xhT = sbX.tile([EMB, BLC], bf16, name="xhT")
            xdT = sbX.tile([EMB, BLC], bf16, name="xdT")
            with tc.tile_pool(name="psX", bufs=1, space="PSUM") as psX:
                psh = [psX.tile([EMB, 400], f32, name=f"psxh{i}",
                                tag=f"psxh{i}") for i in range(4)]
                psd = [psX.tile([EMB, 400], f32, name=f"psxd{i}",
                                tag=f"psxd{i}") for i in range(4)]
                for k2 in range(NK // XC):
                    xsb = xstream.tile([KC, XC * BLC], bf16, name="xsb",
                                       tag="xsb")
                    big_dma(xsb[:], xt[:, XC * k2 * BLC:(XC * k2 + XC) * BLC])
                    for c in range(XC):
                        k = XC * k2 + c
                        for i, (off, nn_) in enumerate(XNT):
                            nc.tensor.matmul(psh[i][:],
                                             qh_sb[:, k * EMB:(k + 1) * EMB],
                                             xsb[:, c * BLC + off:c * BLC + off + nn_],
                                             start=(k == 0), stop=(k == NK - 1))
                            nc.tensor.matmul(psd[i][:],
                                             qd_sb[:, k * EMB:(k + 1) * EMB],
                                             xsb[:, c * BLC + off:c * BLC + off + nn_],
                                             start=(k == 0), stop=(k == NK - 1))
                for i, (off, nn_) in enumerate(XNT):
                    nc.vector.tensor_copy(xhT[:, off:off + nn_], psh[i][:])
                    nc.vector.tensor_copy(xdT[:, off:off + nn_], psd[i][:])

            # ============ GRU input projections ============
            with tc.tile_pool(name="psP", bufs=3, space="PSUM") as psP, \
                 tc.tile_pool(name="sbW", bufs=1) as sbW:
                wih_sb, pb_sb, bhhn_sb = [], [], []
                for u in range(2):
                    wt = sbW.tile([EMB, 3 * H], bf16, name=f"wihsb{u}")
                    nc.sync.dma_start(wt[:], wihT[u][:])
                    wih_sb.append(wt)
                    pb = sbW.tile([EMB, 3], f32, name=f"pbsb{u}")
                    nc.sync.dma_start(pb[:], projb[u][:])
                    pb_sb.append(pb)
                    bh = sbW.tile([EMB, 1], f32, name=f"bhhnsb{u}")
                    nc.sync.dma_start(bh[:], bhhn[u][:])
                    bhhn_sb.append(bh)
                zsrc = sbW.tile([EMB, BLC], bf16, name="zsrc")
                nc.gpsimd.memset(zsrc[:], 0.0)
                # bhh_n bias columns (constant over t, b)
                for u in range(2):
                    nc.scalar.activation(
                        xp_t[:, :, 32 + 8 * u:40 + 8 * u],
                        zsrc.rearrange("p (t b) -> p t b", b=BLOC),
                        AFT.Identity, bias=bhhn_sb[u][:])
                # gate input projections: g=0 (r) -> cols 8u..; g=1 (z) ->
                # 16+8u..; g=2 (n) -> 48+8u..
                gcol = [0, 16, 48]
                for u in range(2):
                    src = xhT if u == 0 else xdT
                    for g in range(3):
                        for nt in range(4):
                            ps = psP.tile([EMB, 400], f32, name="psP",
                                          tag="psP")
                            nc.tensor.matmul(
                                ps[:], wih_sb[u][:, g * H:(g + 1) * H],
                                src[:, nt * 400:(nt + 1) * 400],
                                start=True, stop=True)
                            nc.scalar.activation(
                                xp_t[:, nt * 50:(nt + 1) * 50,
                                     gcol[g] + 8 * u:gcol[g] + 8 * u + 8],
                                ps.rearrange("p (t b) -> p t b", b=BLOC),
                                AFT.Identity, bias=pb_sb[u][:, g:g + 1])

        # ================= GRU + heads phase =================
        with tc.tile_pool(name="sbR", bufs=1) as sbR, \
             tc.tile_pool(name="sbh", bufs=2) as sbh, \
             tc.tile_pool(name="sbstep", bufs=4) as sbs, \
             tc.tile_pool(name="stg", bufs=2) as stg, \
             tc.tile_pool(name="psG", bufs=4, space="PSUM") as psG, \
             tc.tile_pool(name="psTh", bufs=1, space="PSUM") as psTh, \
             tc.tile_pool(name="psH", bufs=2, space="PSUM") as psH:
            whh_sb = []
            for u in range(2):
                wt = sbR.tile([EMB, 3 * H], bf16, name=f"whhsb{u}")
                nc.sync.dma_start(wt[:], whhT[u][:])
                whh_sb.append(wt)
            w1w_sb = sbR.tile([EMB, EMB], bf16, name="w1wsb")
            nc.sync.dma_start(w1w_sb[:], w1wT[:])
            w2w_sb = sbR.tile([EMB, EMB], bf16, name="w2wsb")
            nc.sync.dma_start(w2w_sb[:], w2wT[:])
            wb_sb = sbR.tile([EMB, 1], f32, name="wbsb")
            nc.sync.dma_start(wb_sb[:], wb[:])
            hw_sb = {}
            for nm, t_ in (("fcc", fccwT), ("fct", fctwT)):
                w_ = sbR.tile([EMB, Q], bf16, name=f"{nm}wsb")
                nc.gpsimd.dma_start(w_[:], t_[:])
                hw_sb[nm] = w_
            fce0 = sbR.tile([EMB, Q], bf16, name="fce0sb")
            nc.gpsimd.dma_start(fce0[:], fcewT[0:EMB, :])
            fce1 = sbR.tile([EMB, Q], bf16, name="fce1sb")
            nc.gpsimd.dma_start(fce1[:], fcewT[EMB:2 * EMB, :])

            outT = sbR.tile([EMB, L * 16], bf16, name="outT")
            outT_v = outT.rearrange("p (t u b) -> p t u b", u=2, b=BLOC)
            zero16_f = sbR.tile([EMB, 16], f32, name="zero16_f")
            nc.gpsimd.memset(zero16_f[:], 0.0)
            zero16 = sbR.tile([EMB, 16], bf16, name="zero16")
            nc.vector.tensor_copy(zero16[:], zero16_f[:])
            stag = {nm: stg.tile([128, Q], bf16, name=f"stag_{nm}")
                    for nm in ("c", "t", "e")}
            out_flat = {"c": out_c.rearrange("l b q -> (l b) q"),
                        "t": out_t.rearrange("l b q -> (l b) q"),
                        "e": out_e.rearrange("l b q -> (l b) q")}

            def head_chunk(j, nt16):
                rows = nt16 * BLOC
                lh = sbh.tile([EMB, 128], bf16, name="lh", tag="lh")
                ld = sbh.tile([EMB, 128], bf16, name="ld", tag="ld")
                nc.vector.tensor_copy(
                    lh[:, :rows].rearrange("p (t b) -> p t b", b=BLOC),
                    outT_v[:, 16 * j:16 * j + nt16, 0, :])
                nc.vector.tensor_copy(
                    ld[:, :rows].rearrange("p (t b) -> p t b", b=BLOC),
                    outT_v[:, 16 * j:16 * j + nt16, 1, :])
                pst = psTh.tile([EMB, 128], f32, name="pstheta", tag="pstheta")
                nc.tensor.matmul(pst[:, :rows], w1w_sb[:], lh[:, :rows],
                                 start=True, stop=False)
                nc.tensor.matmul(pst[:, :rows], w2w_sb[:], ld[:, :rows],
                                 start=False, stop=True)
                theta = sbh.tile([EMB, 128], bf16, name="theta", tag="theta")
                nc.scalar.activation(theta[:, :rows], pst[:, :rows],
                                     AFT.Sigmoid, bias=wb_sb[:])
                omt = sbh.tile([EMB, 128], bf16, name="omt", tag="omt")
                nc.scalar.activation(omt[:, :rows], theta[:, :rows],
                                     AFT.Identity, scale=-1.0, bias=1.0)
                od = sbh.tile([EMB, 128], bf16, name="od", tag="od")
                nc.vector.tensor_mul(od[:, :rows], theta[:, :rows],
                                     ld[:, :rows])
                oh = sbh.tile([EMB, 128], bf16, name="oh", tag="oh")
                nc.vector.tensor_mul(oh[:, :rows], omt[:, :rows],
                                     lh[:, :rows])
                for noff, nsz in HNT:
                    psc = psH.tile([128, 512], f32, name="psc", tag="psh")
                    nc.tensor.matmul(psc[:rows, :nsz], lh[:, :rows],
                                     hw_sb["fcc"][:, noff:noff + nsz],
                                     start=True, stop=True)
                    nc.scalar.activation(
                        stag["c"][:rows, noff:noff + nsz], psc[:rows, :nsz],
                        AFT.Identity)
                    psc = psH.tile([128, 512], f32, name="psc2", tag="psh")
                    nc.tensor.matmul(psc[:rows, :nsz], ld[:, :rows],
                                     hw_sb["fct"][:, noff:noff + nsz],
                                     start=True, stop=True)
                    nc.scalar.activation(
                        stag["t"][:rows, noff:noff + nsz], psc[:rows, :nsz],
                        AFT.Identity)
                    psc = psH.tile([128, 512], f32, name="psc3", tag="psh")
                    nc.tensor.matmul(psc[:rows, :nsz], od[:, :rows],
                                     fce0[:, noff:noff + nsz],
                                     start=True, stop=False)
                    nc.tensor.matmul(psc[:rows, :nsz], oh[:, :rows],
                                     fce1[:, noff:noff + nsz],
                                     start=False, stop=True)
                    nc.vector.tensor_copy(
                        stag["e"][:rows, noff:noff + nsz], psc[:rows, :nsz])
                for nm in ("c", "t", "e"):
                    big_dma(out_flat[nm][128 * j:128 * j + rows, :],
                            stag[nm][:rows, :])

            # GRU recurrence: per step one PSUM tile [128,48] with columns
            #   0:8 r_u0 | 8:16 r_u1 | 16:24 z_u0 | 24:32 z_u1
            #   32:40 n_u0 | 40:48 n_u1
            # filled by ident-matmul of xp cols 0:48 (r/z projections + bhh_n).
            # h is kept SPLIT as h = zh + zbn with zh = z*h_prev and
            # zbn = (1-z)*n; the Whh matmuls take both as moving operands and
            # PSUM adds them, so the h-recombine (outT write, for the heads)
            # leaves the recurrence critical path.
            ones16 = sbR.tile([EMB, 16], bf16, name="ones16")
            nc.gpsimd.memset(ones16[:], 1.0)
            zh_p, zbn_p = zero16, zero16
            for t in range(L):
                ps = psG.tile([EMB, 48], f32, name="psg", tag="psg")
                nc.tensor.matmul(ps[:], ident_b[:], xp_t[:, t, 0:48],
                                 start=True, stop=False)
                # r/z matmuls first so the sigmoid fires 4 MM slots earlier;
                # n matmuls after (their consumer rn waits on the sigmoid
                # anyway).
                for part, last in ((zh_p, False), (zbn_p, True)):
                    for u in range(2):
                        hp = part[:, 8 * u:8 * u + 8]
                        nc.tensor.matmul(ps[:, 8 * u:8 * u + 8],
                                         whh_sb[u][:, 0:H], hp,
                                         start=False, stop=last)
                        nc.tensor.matmul(ps[:, 16 + 8 * u:24 + 8 * u],
                                         whh_sb[u][:, H:2 * H], hp,
                                         start=False, stop=last)
                for part, last in ((zh_p, False), (zbn_p, True)):
                    for u in range(2):
                        hp = part[:, 8 * u:8 * u + 8]
                        nc.tensor.matmul(ps[:, 32 + 8 * u:40 + 8 * u],
                                         whh_sb[u][:, 2 * H:3 * H], hp,
                                         start=False, stop=last)
                gates = sbs.tile([EMB, 32], bf16, name="gates", tag="gates")
                nc.scalar.activation(gates[:], ps[:, 0:32], AFT.Sigmoid)
                rn = sbs.tile([EMB, 16], bf16, name="rn", tag="rn")
                nc.vector.tensor_mul(rn[:], gates[:, 0:16], ps[:, 32:48])
                npre = sbs.tile([EMB, 16], bf16, name="npre", tag="npre")
                nc.vector.tensor_add(npre[:], rn[:], xp_t[:, t, 48:64])
                omz = sbs.tile([EMB, 16], bf16, name="omz", tag="omz")
                nc.scalar.activation(omz[:], gates[:, 16:32], AFT.Identity,
                                     scale=-1.0, bias=1.0)
                nn = sbs.tile([EMB, 16], bf16, name="nn", tag="nn")
                nc.scalar.activation(nn[:], npre[:], AFT.Tanh)
                hprev = (outT[:, 16 * (t - 1):16 * (t - 1) + 16]
                         if t > 0 else zero16[:])
                zh = sbs.tile([EMB, 16], bf16, name="zh", tag="zh")
                nc.vector.tensor_mul(zh[:], gates[:, 16:32], hprev)
                zbn = sbs.tile([EMB, 16], bf16, name="zbn", tag="zbn")
                nc.vector.tensor_mul(zbn[:], omz[:], nn[:])
                nc.vector.tensor_add(outT[:, 16 * t:16 * t + 16],
                                     zh[:], zbn[:])
                zh_p, zbn_p = zh, zbn
            # heads emitted after the loop: lower scheduler priority, so the
            # recurrence chain never waits behind head matmuls
            for j in range(12):
                head_chunk(j, 16)
            head_chunk(12, 8)  # last 64 rows (t in [192,200))

    nc.compile()
    return nc


def _bf(a):
    return np.ascontiguousarray(np.asarray(a, np.float32)).astype(
        ml_dtypes.bfloat16)


def _pack_k(a):
    """[NQ, W] f32 -> [KC, NK*W] bf16 with row (k*KC+p) -> (p, k)."""
    w = a.shape[1]
    return np.ascontiguousarray(
        a.reshape(NK, KC, w).transpose(1, 0, 2).reshape(KC, NK * w)).astype(
            ml_dtypes.bfloat16)


def _host_prep(inputs):
    """Build the 8 per-core input maps from the full problem inputs."""
    f = np.float32
    x = np.asarray(inputs["x"], f)
    ques = np.asarray(inputs["ques"], f)

    def T(a):
        return np.ascontiguousarray(np.asarray(a, f).T)

    # layer-1 GCN activations, computed on host (tiny)
    z1 = {"hg": ques @ inputs["hg_W1"] + inputs["hg_b1"],
          "g1": ques @ inputs["g1_W1"] + inputs["g1_b1"],
          "g2": ques @ inputs["g2_W1"] + inputs["g2_b1"]}
    graphs = {"hg": inputs["G"], "g1": inputs["adj_out"], "g2": inputs["adj_in"]}

    shared = {
        "z1_hg": _pack_k(np.asarray(z1["hg"], f)),
        "z1_g1": _pack_k(np.asarray(z1["g1"], f)),
        "z1_g2": _pack_k(np.asarray(z1["g2"], f)),
        "w2_hg": _bf(inputs["hg_W2"]),
        "w2_g1": _bf(inputs["g1_W2"]),
        "w2_g2": _bf(inputs["g2_W2"]),
        "b2_hg": _bf(np.asarray(inputs["hg_b2"], f).reshape(1, -1)),
        "b2_g1": _bf(np.asarray(inputs["g1_b2"], f).reshape(1, -1)),
        "b2_g2": _bf(np.asarray(inputs["g2_b2"], f).reshape(1, -1)),
        "wihT1": _bf(T(inputs["r1_Wih"])),
        "wihT2": _bf(T(inputs["r2_Wih"])),
        "whhT1": _bf(T(inputs["r1_Whh"])),
        "whhT2": _bf(T(inputs["r2_Whh"])),
        "w1wT": _bf(T(inputs["w1_W"])),
        "w2wT": _bf(T(inputs["w2_W"])),
        "wb": np.asarray(inputs["w1_b"] + inputs["w2_b"], f).reshape(-1, 1),
        "fccwT": _bf(T(inputs["fcc_W"])),
        "fctwT": _bf(T(inputs["fct_W"])),
        "fcewT": _bf(T(inputs["fce_W"])),
    }
    for u, (ih, hh) in enumerate((("r1_bih", "r1_bhh"), ("r2_bih", "r2_bhh"))):
        bih = np.asarray(inputs[ih], f)
        bhh = np.asarray(inputs[hh], f)
        pb = np.zeros((EMB, 3), f)
        for g in range(3):
            pb[:, g] = bih[g * H:(g + 1) * H]
            if g < 2:  # r, z: fold bhh into the projection bias
                pb[:, g] += bhh[g * H:(g + 1) * H]
        shared[f"projb{u + 1}"] = pb
        shared[f"bhhn{u + 1}"] = bhh[2 * H:3 * H].reshape(-1, 1).copy()

    # convert each A^T once, slice per core
    atT = {}
    for g, arr in graphs.items():
        atT[g] = np.asarray(arr, f).T.astype(ml_dtypes.bfloat16)

    in_maps = []
    for c in range(NCORES):
        m = dict(shared)
        for g in graphs:
            atc = np.zeros((NQ, SHARD_P), ml_dtypes.bfloat16)
            atc[:, :SHARD] = atT[g][:, c * SHARD:(c + 1) * SHARD]
            m[f"at_{g}"] = atc
        xc = x[c * BLOC:(c + 1) * BLOC]           # [8, 200, 5000]
        xts = xc.transpose(2, 1, 0).reshape(NQ, BLC)  # col = t*8 + b
        m["xt"] = np.ascontiguousarray(
            xts.reshape(NK, KC, BLC).transpose(1, 0, 2)
            .reshape(KC, NK * BLC)).astype(ml_dtypes.bfloat16)
        in_maps.append(m)
    return in_maps


def kernel(**inputs):
    global _BUILT, LAST
    from concourse import bass_utils
    if _BUILT is None:
        _BUILT = _build(debug=False)
    nc = _BUILT
    in_maps = _host_prep(inputs)
    res = bass_utils.run_bass_kernel_spmd(nc, in_maps,
                                          core_ids=list(range(NCORES)))
    LAST = res
    f = np.float32
    logit_c = np.empty((B, L, Q), f)
    logit_t = np.empty((B, L, Q), f)
    logit_e = np.empty((B, L, Q), f)
    for c in range(NCORES):
        r = res.results[c]
        logit_c[c * BLOC:(c + 1) * BLOC] = \
            np.asarray(r["out_c"], dtype=f).transpose(1, 0, 2)
        logit_t[c * BLOC:(c + 1) * BLOC] = \
            np.asarray(r["out_t"], dtype=f).transpose(1, 0, 2)
        logit_e[c * BLOC:(c + 1) * BLOC] = \
            np.asarray(r["out_e"], dtype=f).transpose(1, 0, 2)
    for arr, bname in ((logit_c, "fcc_b"), (logit_t, "fct_b"),
                       (logit_e, "fce_b")):
        bias = np.asarray(inputs[bname], f)
        if np.any(bias):
            arr += bias
    return (logit_c, logit_t, logit_e)


# revision 10
# speedup vs baseline: 1.0219x; 1.0219x over previous
"""Trainium2 Bass kernel for the DKT (graph-based knowledge tracing) model.

Sharding across the 8 NeuronCores:
  - GCN phase: row-shard of the three [5000,5000] adjacency matmuls (625 rows
    per core), with AllGathers of the small [5000,EMB] intermediates.
  - x@ques / GRU / logit heads: data-parallel over batch (8 sequences/core).

Everything large is bf16 (rel-err budget 2e-2; measured ~1e-3): halves HBM
traffic (the kernel is memory-bound) and enables fast-weight-load on the PE
for the GRU's tiny recurrence matmuls. PSUM accumulation stays fp32.

Layouts are host-packed so every big DMA is contiguous per partition:
  - at_g   [NQ, 640]        A^T column-shard, padded 625->640
  - z1p_g  [125, 40*128]    layer-1 GCN activations, contraction-chunk packed
  - xtp    [125, 40*1600]   x^T batch-shard, contraction-chunk packed
GRU per step t uses one PSUM tile [128,48] = [r0 r1 z0 z1 n0 n1] built by one
identity-matmul (folds the precomputed input projections + bhh_n bias) plus
six bf16 Whh matmuls; gates then take 2 scalar + 5 vector ops per step for
both GRUs together.
"""

import numpy as np
import ml_dtypes

Q = 2500
NQ = 5000
EMB = 128
H = 128
B = 64
L = 200
NCORES = 8
SHARD = NQ // NCORES          # 625 adjacency rows per core
KC = 125                      # contraction chunk (partition dim)
NK = NQ // KC                 # 40 chunks
BLOC = B // NCORES            # 8 sequences per core
BLC = L * BLOC                # 1600 (bl index = t*8 + b, t-major)
SHARD_P = 640                 # shard padded to even halves
NH = [(0, 320), (320, 320)]   # padded-shard column halves
AC = 10                       # A-rows chunks per DMA (10*125 rows = 1.6MB bf16)
XC = 4                        # xt k-chunks per DMA (1.6MB bf16)
XNT = [(i * 400, 400) for i in range(4)]               # x-stage N tiles
HNT = [(0, 512), (512, 512), (1024, 512), (1536, 512), (2048, 452)]
XPW = 64                      # xp columns per step (see layout below)

_BUILT = None
LAST = None


def _build(debug=False):
    import concourse.bass as bass  # noqa: F401
    import concourse.tile as tile
    from concourse import bacc, mybir
    from concourse.masks import make_identity
    from contextlib import ExitStack

    f32 = mybir.dt.float32
    bf16 = mybir.dt.bfloat16
    AFT = mybir.ActivationFunctionType
    ALU = mybir.AluOpType

    nc = bacc.Bacc("TRN2", target_bir_lowering=False, debug=False,
                   num_devices=NCORES)

    def din(name, shape, dt=bf16):
        return nc.dram_tensor(name, shape, dt, kind="ExternalInput").ap()

    def dout(name, shape, dt=bf16):
        return nc.dram_tensor(name, shape, dt, kind="ExternalOutput").ap()

    # --- inputs (per-core unless noted) ---
    at = {g: din(f"at_{g}", [KC, NK * SHARD_P]) for g in ("hg", "g1", "g2")}
    xt = din("xt", [KC, NK * BLC])
    z1 = {g: din(f"z1_{g}", [KC, NK * EMB]) for g in ("hg", "g1", "g2")}
    e2s = {"hg": EMB, "g1": EMB // 2, "g2": EMB // 2}
    w2 = {g: din(f"w2_{g}", [EMB, e2s[g]]) for g in ("hg", "g1", "g2")}
    b2 = {g: din(f"b2_{g}", [1, e2s[g]]) for g in ("hg", "g1", "g2")}
    wihT = [din("wihT1", [EMB, 3 * H]), din("wihT2", [EMB, 3 * H])]
    whhT = [din("whhT1", [EMB, 3 * H]), din("whhT2", [EMB, 3 * H])]
    projb = [din("projb1", [EMB, 3], f32), din("projb2", [EMB, 3], f32)]
    bhhn = [din("bhhn1", [EMB, 1], f32), din("bhhn2", [EMB, 1], f32)]
    w1wT = din("w1wT", [EMB, EMB])
    w2wT = din("w2wT", [EMB, EMB])
    wb = din("wb", [EMB, 1], f32)
    fccwT = din("fccwT", [EMB, Q])
    fctwT = din("fctwT", [EMB, Q])
    fcewT = din("fcewT", [2 * EMB, Q])

    out_c = dout("out_c", [L, BLOC, Q])
    out_t = dout("out_t", [L, BLOC, Q])
    out_e = dout("out_e", [L, BLOC, Q])

    with tile.TileContext(nc) as tc, ExitStack() as ctx:
        const = ctx.enter_context(tc.tile_pool(name="const", bufs=1))
        dram = ctx.enter_context(tc.tile_pool(name="dram", bufs=1, space="DRAM"))

        ident = const.tile([128, 128], f32, name="ident")
        make_identity(nc, ident[:])
        ident_b = const.tile([128, 128], bf16, name="ident_b")
        nc.vector.tensor_copy(ident_b[:], ident[:])
        ones_f = const.tile([1, 128], f32, name="ones_f")
        nc.gpsimd.memset(ones_f[:], 1.0)
        ones = const.tile([1, 128], bf16, name="ones")
        nc.vector.tensor_copy(ones[:], ones_f[:])

        # DRAM bounce buffers for the AllGathers
        zb = {"hg": dram.tile([SHARD, EMB], bf16, name="zb_hg"),
              "pr": dram.tile([SHARD, EMB], bf16, name="zb_pr")}
        zf = {"hg": dram.tile([NQ, EMB], bf16, name="zf_hg", addr_space="Shared"),
              "pr": dram.tile([NQ, EMB], bf16, name="zf_pr", addr_space="Shared")}
        qb = {"hg": dram.tile([SHARD, EMB], bf16, name="qb_hg"),
              "pr": dram.tile([SHARD, EMB], bf16, name="qb_pr")}
        qf = {"hg": dram.tile([NQ, EMB], bf16, name="qf_hg", addr_space="Shared"),
              "pr": dram.tile([NQ, EMB], bf16, name="qf_pr", addr_space="Shared")}
        RG = [list(range(NCORES))]

        def allgather(inb, outb):
            nc.gpsimd.collective_compute(
                "AllGather", ALU.bypass, replica_groups=RG,
                ins=[inb.opt()], outs=[outb.opt()])

        def rearr_kpe(ap, e):
            return ap.rearrange("(k p) e -> p k e", p=KC)

        # Three independent DMA queues on TRN2: gpsimd (SWDGE, sprays all 16
        # SDMA engines) + the sync/scalar HWDGE queues (both drive engines
        # 0-4 only). Weight the round-robin heavily toward gpsimd so engines
        # 0-4 don't saturate, while the HWDGE queues still break the
        # one-DMA-at-a-time serialization of the single SWDGE queue.
        _dmaq = [nc.gpsimd, nc.gpsimd, nc.gpsimd, nc.gpsimd, nc.sync,
                 nc.gpsimd, nc.gpsimd, nc.gpsimd, nc.gpsimd, nc.scalar]
        _dma_ctr = [0]

        def big_dma(out, in_):
            q = _dmaq[_dma_ctr[0] % len(_dmaq)]
            _dma_ctr[0] += 1
            q.dma_start(out, in_)

        # ================= GCN phase =================
        sbQ = ctx.enter_context(tc.tile_pool(name="sbQ", bufs=1))
        qh_sb = sbQ.tile([KC, NK * EMB], bf16, name="qh_sb")
        qd_sb = sbQ.tile([KC, NK * EMB], bf16, name="qd_sb")
        with tc.tile_pool(name="sbG", bufs=1) as sbG, \
             tc.tile_pool(name="astream", bufs=3) as astream, \
             tc.tile_pool(name="psA", bufs=4, space="PSUM") as psA, \
             tc.tile_pool(name="psW", bufs=2, space="PSUM") as psW, \
             tc.tile_pool(name="psT", bufs=2, space="PSUM") as psT:

            z1sb, hT, w2sb, b2sb = {}, {}, {}, {}
            zstag, qstag, z2f = {}, {}, {}

            def gcn_stage1(g):
                z1sb[g] = sbG.tile([KC, NK * EMB], bf16, name=f"z1sb_{g}",
                                   tag="z1sb", bufs=2)
                nc.gpsimd.dma_start(z1sb[g][:], z1[g][:])
                w2sb[g] = sbG.tile([EMB, e2s[g]], bf16, name=f"w2sb_{g}")
                nc.sync.dma_start(w2sb[g][:], w2[g][:])
                b2sb[g] = sbG.tile([1, e2s[g]], bf16, name=f"b2sb_{g}")
                nc.sync.dma_start(b2sb[g][:], b2[g][:])
                hT[g] = sbG.tile([EMB, SHARD_P], bf16, name=f"hT_{g}",
                                 tag="hT", bufs=2)

                ps = [psA.tile([EMB, 512], f32, name=f"ps1_{g}{i}", tag="psA")
                      for i in range(2)]
                for k4 in range(NK // AC):
                    a_t = astream.tile([KC, AC * SHARD_P], bf16, name="a_t",
                                       tag="a")
                    big_dma(a_t[:], at[g][:, AC * k4 * SHARD_P:
                                          (AC * k4 + AC) * SHARD_P])
                    for c in range(AC):
                        k = AC * k4 + c
                        for i, (off, nh) in enumerate(NH):
                            nc.tensor.matmul(
                                ps[i][:, :nh],
                                z1sb[g][:, k * EMB:(k + 1) * EMB],
                                a_t[:, c * SHARD_P + off:c * SHARD_P + off + nh],
                                start=(k == 0), stop=(k == NK - 1))
                for i, (off, nh) in enumerate(NH):
                    nc.scalar.activation(hT[g][:, off:off + nh],
                                         ps[i][:EMB, :nh], AFT.Relu)

            def gcn_stage2w(g, grp, coloff):
                # Z2 = h @ W2 + b2 (natural layout, into the AG input staging)
                e2 = e2s[g]
                if grp not in zstag:
                    zstag[grp] = sbG.tile([KC, 5 * EMB], bf16,
                                          name=f"zstag_{grp}")
                for c in range(5):
                    ps = psW.tile([KC, EMB], f32, name="psW", tag="psW")
                    nc.tensor.matmul(ps[:, :e2], hT[g][:, c * KC:(c + 1) * KC],
                                     w2sb[g][:], start=True, stop=False)
                    nc.tensor.matmul(ps[:, :e2], ones[:, :KC], b2sb[g][:],
                                     start=False, stop=True)
                    nc.vector.tensor_copy(
                        zstag[grp][:, c * EMB + coloff: c * EMB + coloff + e2],
                        ps[:, :e2])

            def ag_z(grp):
                nc.sync.dma_start(
                    zb[grp].rearrange("(c p) e -> p c e", p=KC),
                    zstag[grp].rearrange("p (c e) -> p c e", c=5))
                allgather(zb[grp], zf[grp])
                z2f[grp] = sbG.tile([KC, NK * EMB], bf16, name=f"z2f_{grp}",
                                    tag="z2f", bufs=2)
                nc.gpsimd.dma_start(
                    z2f[grp].rearrange("p (k e) -> p k e", k=NK),
                    rearr_kpe(zf[grp], EMB))

            def gcn_stage2a(g, grp, coloff):
                e2 = e2s[g]
                o2T = sbG.tile([e2, SHARD_P], f32, name=f"o2T_{g}", tag="o2T",
                               bufs=2)
                ps = [psA.tile([EMB, 512], f32, name=f"ps2_{g}{i}", tag="psA")
                      for i in range(2)]
                for k4 in range(NK // AC):
                    a_t = astream.tile([KC, AC * SHARD_P], bf16, name="a_t2",
                                       tag="a")
                    big_dma(a_t[:], at[g][:, AC * k4 * SHARD_P:
                                          (AC * k4 + AC) * SHARD_P])
                    for c in range(AC):
                        k = AC * k4 + c
                        for i, (off, nh) in enumerate(NH):
                            nc.tensor.matmul(
                                ps[i][:e2, :nh],
                                z2f[grp][:, k * EMB + coloff: k * EMB + coloff + e2],
                                a_t[:, c * SHARD_P + off:c * SHARD_P + off + nh],
                                start=(k == 0), stop=(k == NK - 1))
                for i, (off, nh) in enumerate(NH):
                    nc.vector.tensor_copy(o2T[:, off:off + nh], ps[i][:e2, :nh])
                # transpose to natural layout; stage for the output AllGather
                if grp not in qstag:
                    qstag[grp] = sbG.tile([KC, 5 * EMB], bf16,
                                          name=f"qstag_{grp}")
                # ques_d = concat([ques_in(g2), ques_out(g1)]): g2 -> cols
                # 0:64, g1 -> cols 64:128 of each block; hg -> full block.
                qoff = {"hg": 0, "g1": 64, "g2": 0}[g]
                for c in range(5):
                    pst = psT.tile([KC, EMB], f32, name="psT", tag="psT")
                    nc.tensor.transpose(pst[:, :e2],
                                        o2T[:, c * KC:(c + 1) * KC],
                                        ident[:e2, :e2])
                    nc.vector.tensor_copy(
                        qstag[grp][:, c * EMB + qoff: c * EMB + qoff + e2],
                        pst[:, :e2])

            def ag_q(grp):
                nc.sync.dma_start(
                    qb[grp].rearrange("(c p) e -> p c e", p=KC),
                    qstag[grp].rearrange("p (c e) -> p c e", c=5))
                allgather(qb[grp], qf[grp])

            gcn_stage1("hg")
            gcn_stage2w("hg", "hg", 0)
            ag_z("hg")
            gcn_stage1("g1")
            gcn_stage2w("g1", "pr", 0)
            gcn_stage1("g2")
            gcn_stage2w("g2", "pr", 64)
            ag_z("pr")
            gcn_stage2a("hg", "hg", 0)
            ag_q("hg")
            nc.gpsimd.dma_start(qh_sb.rearrange("p (k e) -> p k e", k=NK),
                                rearr_kpe(qf["hg"], EMB))
            gcn_stage2a("g1", "pr", 0)
            gcn_stage2a("g2", "pr", 64)
            ag_q("pr")
            nc.gpsimd.dma_start(qd_sb.rearrange("p (k e) -> p k e", k=NK),
                                rearr_kpe(qf["pr"], EMB))

        # ================= x @ ques phase =================
        # xp layout per step t (XPW=64 cols):
        #   0:8 xr_u0 | 8:16 xr_u1 | 16:24 xz_u0 | 24:32 xz_u1
        #   32:40 bhhn_u0 | 40:48 bhhn_u1 | 48:56 xn_u0 | 56:64 xn_u1
        sbP = ctx.enter_context(tc.tile_pool(name="sbP", bufs=1))
        xp = sbP.tile([EMB, L * XPW], bf16, name="xp")
        xp_t = xp.rearrange("p (t c) -> p t c", c=XPW)

        with tc.tile_pool(name="sbX", bufs=1) as sbX, \
             tc.tile_pool(name="xstream", bufs=3) as xstream:
            xhT = sbX.tile([EMB, BLC], bf16, name="xhT")
            xdT = sbX.tile([EMB, BLC], bf16, name="xdT")
            with tc.tile_pool(name="psX", bufs=1, space="PSUM") as psX:
                psh = [psX.tile([EMB, 400], f32, name=f"psxh{i}",
                                tag=f"psxh{i}") for i in range(4)]
                psd = [psX.tile([EMB, 400], f32, name=f"psxd{i}",
                                tag=f"psxd{i}") for i in range(4)]
                for k2 in range(NK // XC):
                    xsb = xstream.tile([KC, XC * BLC], bf16, name="xsb",
                                       tag="xsb")
                    big_dma(xsb[:], xt[:, XC * k2 * BLC:(XC * k2 + XC) * BLC])
                    for c in range(XC):
                        k = XC * k2 + c
                        for i, (off, nn_) in enumerate(XNT):
                            nc.tensor.matmul(psh[i][:],
                                             qh_sb[:, k * EMB:(k + 1) * EMB],
                                             xsb[:, c * BLC + off:c * BLC + off + nn_],
                                             start=(k == 0), stop=(k == NK - 1))
                            nc.tensor.matmul(psd[i][:],
                                             qd_sb[:, k * EMB:(k + 1) * EMB],
                                             xsb[:, c * BLC + off:c * BLC + off + nn_],
                                             start=(k == 0), stop=(k == NK - 1))
                for i, (off, nn_) in enumerate(XNT):
                    nc.vector.tensor_copy(xhT[:, off:off + nn_], psh[i][:])
                    nc.vector.tensor_copy(xdT[:, off:off + nn_], psd[i][:])

            # ============ GRU input projections ============
            with tc.tile_pool(name="psP", bufs=3, space="PSUM") as psP, \
                 tc.tile_pool(name="sbW", bufs=1) as sbW:
                wih_sb, pb_sb, bhhn_sb = [], [], []
                for u in range(2):
                    wt = sbW.tile([EMB, 3 * H], bf16, name=f"wihsb{u}")
                    nc.sync.dma_start(wt[:], wihT[u][:])
                    wih_sb.append(wt)
                    pb = sbW.tile([EMB, 3], f32, name=f"pbsb{u}")
                    nc.sync.dma_start(pb[:], projb[u][:])
                    pb_sb.append(pb)
                    bh = sbW.tile([EMB, 1], f32, name=f"bhhnsb{u}")
                    nc.sync.dma_start(bh[:], bhhn[u][:])
                    bhhn_sb.append(bh)
                zsrc = sbW.tile([EMB, BLC], bf16, name="zsrc")
                nc.gpsimd.memset(zsrc[:], 0.0)
                # bhh_n bias columns (constant over t, b)
                for u in range(2):
                    nc.scalar.activation(
                        xp_t[:, :, 32 + 8 * u:40 + 8 * u],
                        zsrc.rearrange("p (t b) -> p t b", b=BLOC),
                        AFT.Identity, bias=bhhn_sb[u][:])
                # gate input projections: g=0 (r) -> cols 8u..; g=1 (z) ->
                # 16+8u..; g=2 (n) -> 48+8u..
                gcol = [0, 16, 48]
                for u in range(2):
                    src = xhT if u == 0 else xdT
                    for g in range(3):
                        for nt in range(4):
                            ps = psP.tile([EMB, 400], f32, name="psP",
                                          tag="psP")
                            nc.tensor.matmul(
                                ps[:], wih_sb[u][:, g * H:(g + 1) * H],
                                src[:, nt * 400:(nt + 1) * 400],
                                start=True, stop=True)
                            nc.scalar.activation(
                                xp_t[:, nt * 50:(nt + 1) * 50,
                                     gcol[g] + 8 * u:gcol[g] + 8 * u + 8],
                                ps.rearrange("p (t b) -> p t b", b=BLOC),
                                AFT.Identity, bias=pb_sb[u][:, g:g + 1])

        # ================= GRU + heads phase =================
        with tc.tile_pool(name="sbR", bufs=1) as sbR, \
             tc.tile_pool(name="sbh", bufs=2) as sbh, \
             tc.tile_pool(name="sbstep", bufs=4) as sbs, \
             tc.tile_pool(name="stg", bufs=2) as stg, \
             tc.tile_pool(name="psG", bufs=4, space="PSUM") as psG, \
             tc.tile_pool(name="psTh", bufs=1, space="PSUM") as psTh, \
             tc.tile_pool(name="psH", bufs=2, space="PSUM") as psH:
            whh_sb = []
            for u in range(2):
                wt = sbR.tile([EMB, 3 * H], bf16, name=f"whhsb{u}")
                nc.sync.dma_start(wt[:], whhT[u][:])
                whh_sb.append(wt)
            w1w_sb = sbR.tile([EMB, EMB], bf16, name="w1wsb")
            nc.sync.dma_start(w1w_sb[:], w1wT[:])
            w2w_sb = sbR.tile([EMB, EMB], bf16, name="w2wsb")
            nc.sync.dma_start(w2w_sb[:], w2wT[:])
            wb_sb = sbR.tile([EMB, 1], f32, name="wbsb")
            nc.sync.dma_start(wb_sb[:], wb[:])
            hw_sb = {}
            for nm, t_ in (("fcc", fccwT), ("fct", fctwT)):
                w_ = sbR.tile([EMB, Q], bf16, name=f"{nm}wsb")
                nc.gpsimd.dma_start(w_[:], t_[:])
                hw_sb[nm] = w_
            fce0 = sbR.tile([EMB, Q], bf16, name="fce0sb")
            nc.gpsimd.dma_start(fce0[:], fcewT[0:EMB, :])
            fce1 = sbR.tile([EMB, Q], bf16, name="fce1sb")
            nc.gpsimd.dma_start(fce1[:], fcewT[EMB:2 * EMB, :])

            outT = sbR.tile([EMB, L * 16], bf16, name="outT")
            outT_v = outT.rearrange("p (t u b) -> p t u b", u=2, b=BLOC)
            zero16_f = sbR.tile([EMB, 16], f32, name="zero16_f")
            nc.gpsimd.memset(zero16_f[:], 0.0)
            zero16 = sbR.tile([EMB, 16], bf16, name="zero16")
            nc.vector.tensor_copy(zero16[:], zero16_f[:])
            stag = {nm: stg.tile([128, Q], bf16, name=f"stag_{nm}")
                    for nm in ("c", "t", "e")}
            out_flat = {"c": out_c.rearrange("l b q -> (l b) q"),
                        "t": out_t.rearrange("l b q -> (l b) q"),
                        "e": out_e.rearrange("l b q -> (l b) q")}

            def head_chunk(j, nt16):
                rows = nt16 * BLOC
                lh = sbh.tile([EMB, 128], bf16, name="lh", tag="lh")
                ld = sbh.tile([EMB, 128], bf16, name="ld", tag="ld")
                nc.vector.tensor_copy(
                    lh[:, :rows].rearrange("p (t b) -> p t b", b=BLOC),
                    outT_v[:, 16 * j:16 * j + nt16, 0, :])
                nc.vector.tensor_copy(
                    ld[:, :rows].rearrange("p (t b) -> p t b", b=BLOC),
                    outT_v[:, 16 * j:16 * j + nt16, 1, :])
                pst = psTh.tile([EMB, 128], f32, name="pstheta", tag="pstheta")
                nc.tensor.matmul(pst[:, :rows], w1w_sb[:], lh[:, :rows],
                                 start=True, stop=False)
                nc.tensor.matmul(pst[:, :rows], w2w_sb[:], ld[:, :rows],
                                 start=False, stop=True)
                theta = sbh.tile([EMB, 128], bf16, name="theta", tag="theta")
                nc.scalar.activation(theta[:, :rows], pst[:, :rows],
                                     AFT.Sigmoid, bias=wb_sb[:])
                omt = sbh.tile([EMB, 128], bf16, name="omt", tag="omt")
                nc.scalar.activation(omt[:, :rows], theta[:, :rows],
                                     AFT.Identity, scale=-1.0, bias=1.0)
                od = sbh.tile([EMB, 128], bf16, name="od", tag="od")
                nc.vector.tensor_mul(od[:, :rows], theta[:, :rows],
                                     ld[:, :rows])
                oh = sbh.tile([EMB, 128], bf16, name="oh", tag="oh")
                nc.vector.tensor_mul(oh[:, :rows], omt[:, :rows],
                                     lh[:, :rows])
                for noff, nsz in HNT:
                    psc = psH.tile([128, 512], f32, name="psc", tag="psh")
                    nc.tensor.matmul(psc[:rows, :nsz], lh[:, :rows],
                                     hw_sb["fcc"][:, noff:noff + nsz],
                                     start=True, stop=True)
                    nc.scalar.activation(
                        stag["c"][:rows, noff:noff + nsz], psc[:rows, :nsz],
                        AFT.Identity)
                    psc = psH.tile([128, 512], f32, name="psc2", tag="psh")
                    nc.tensor.matmul(psc[:rows, :nsz], ld[:, :rows],
                                     hw_sb["fct"][:, noff:noff + nsz],
                                     start=True, stop=True)
                    nc.scalar.activation(
                        stag["t"][:rows, noff:noff + nsz], psc[:rows, :nsz],
                        AFT.Identity)
                    psc = psH.tile([128, 512], f32, name="psc3", tag="psh")
                    nc.tensor.matmul(psc[:rows, :nsz], od[:, :rows],
                                     fce0[:, noff:noff + nsz],
                                     start=True, stop=False)
                    nc.tensor.matmul(psc[:rows, :nsz], oh[:, :rows],
                                     fce1[:, noff:noff + nsz],
                                     start=False, stop=True)
                    nc.vector.tensor_copy(
                        stag["e"][:rows, noff:noff + nsz], psc[:rows, :nsz])
                for nm in ("c", "t", "e"):
                    big_dma(out_flat[nm][128 * j:128 * j + rows, :],
                            stag[nm][:rows, :])

            # GRU recurrence: per step one PSUM tile [128,48] with columns
            #   0:8 r_u0 | 8:16 r_u1 | 16:24 z_u0 | 24:32 z_u1
            #   32:40 n_u0 | 40:48 n_u1
            # filled by ident-matmul of xp cols 0:48 (r/z projections + bhh_n).
            # h is kept SPLIT as h = zh + zbn with zh = z*h_prev and
            # zbn = (1-z)*n; the Whh matmuls take both as moving operands and
            # PSUM adds them, so the h-recombine (outT write, for the heads)
            # leaves the recurrence critical path.
            ones16 = sbR.tile([EMB, 16], bf16, name="ones16")
            nc.gpsimd.memset(ones16[:], 1.0)
            zh_p, zbn_p = zero16, zero16
            for t in range(L):
                ps = psG.tile([EMB, 48], f32, name="psg", tag="psg")
                nc.tensor.matmul(ps[:], ident_b[:], xp_t[:, t, 0:48],
                                 start=True, stop=False)
                # r/z matmuls first so the sigmoid fires 4 MM slots earlier;
                # n matmuls after (their consumer rn waits on the sigmoid
                # anyway).
                for part, last in ((zh_p, False), (zbn_p, True)):
                    for u in range(2):
                        hp = part[:, 8 * u:8 * u + 8]
                        nc.tensor.matmul(ps[:, 8 * u:8 * u + 8],
                                         whh_sb[u][:, 0:H], hp,
                                         start=False, stop=last)
                        nc.tensor.matmul(ps[:, 16 + 8 * u:24 + 8 * u],
                                         whh_sb[u][:, H:2 * H], hp,
                                         start=False, stop=last)
                for part, last in ((zh_p, False), (zbn_p, True)):
                    for u in range(2):
                        hp = part[:, 8 * u:8 * u + 8]
                        nc.tensor.matmul(ps[:, 32 + 8 * u:40 + 8 * u],
                                         whh_sb[u][:, 2 * H:3 * H], hp,
                                         start=False, stop=last)
                gates = sbs.tile([EMB, 32], bf16, name="gates", tag="gates")
                nc.scalar.activation(gates[:], ps[:, 0:32], AFT.Sigmoid)
                rn = sbs.tile([EMB, 16], bf16, name="rn", tag="rn")
                nc.vector.tensor_mul(rn[:], gates[:, 0:16], ps[:, 32:48])
                npre = sbs.tile([EMB, 16], bf16, name="npre", tag="npre")
                nc.vector.tensor_add(npre[:], rn[:], xp_t[:, t, 48:64])
                omz = sbs.tile([EMB, 16], bf16, name="omz", tag="omz")
                nc.scalar.activation(omz[:], gates[:, 16:32], AFT.Identity,
                                     scale=-1.0, bias=1.0)
                nn = sbs.tile([EMB, 16], bf16, name="nn", tag="nn")
                nc.scalar.activation(nn[:], npre[:], AFT.Tanh)
                hprev = (outT[:, 16 * (t - 1):16 * (t - 1) + 16]
                         if t > 0 else zero16[:])
                zh = sbs.tile([EMB, 16], bf16, name="zh", tag="zh")
                nc.vector.tensor_mul(zh[:], gates[:, 16:32], hprev)
                zbn = sbs.tile([EMB, 16], bf16, name="zbn", tag="zbn")
                nc.vector.tensor_mul(zbn[:], omz[:], nn[:])
                nc.vector.tensor_add(outT[:, 16 * t:16 * t + 16],
                                     zh[:], zbn[:])
                zh_p, zbn_p = zh, zbn
            # heads emitted after the loop: lower scheduler priority, so the
            # recurrence chain never waits behind head matmuls
            for j in range(12):
                head_chunk(j, 16)
            head_chunk(12, 8)  # last 64 rows (t in [192,200))

    nc.compile()
    return nc


def _bf(a):
    return np.ascontiguousarray(np.asarray(a, np.float32)).astype(
        ml_dtypes.bfloat16)


def _pack_k(a):
    """[NQ, W] f32 -> [KC, NK*W] bf16 with row (k*KC+p) -> (p, k)."""
    w = a.shape[1]
    return np.ascontiguousarray(
        a.reshape(NK, KC, w).transpose(1, 0, 2).reshape(KC, NK * w)).astype(
            ml_dtypes.bfloat16)


def _host_prep(inputs):
    """Build the 8 per-core input maps from the full problem inputs."""
    f = np.float32
    x = np.asarray(inputs["x"], f)
    ques = np.asarray(inputs["ques"], f)

    def T(a):
        return np.ascontiguousarray(np.asarray(a, f).T)

    # layer-1 GCN activations, computed on host (tiny)
    z1 = {"hg": ques @ inputs["hg_W1"] + inputs["hg_b1"],
          "g1": ques @ inputs["g1_W1"] + inputs["g1_b1"],
          "g2": ques @ inputs["g2_W1"] + inputs["g2_b1"]}
    graphs = {"hg": inputs["G"], "g1": inputs["adj_out"], "g2": inputs["adj_in"]}

    shared = {
        "z1_hg": _pack_k(np.asarray(z1["hg"], f)),
        "z1_g1": _pack_k(np.asarray(z1["g1"], f)),
        "z1_g2": _pack_k(np.asarray(z1["g2"], f)),
        "w2_hg": _bf(inputs["hg_W2"]),
        "w2_g1": _bf(inputs["g1_W2"]),
        "w2_g2": _bf(inputs["g2_W2"]),
        "b2_hg": _bf(np.asarray(inputs["hg_b2"], f).reshape(1, -1)),
        "b2_g1": _bf(np.asarray(inputs["g1_b2"], f).reshape(1, -1)),
        "b2_g2": _bf(np.asarray(inputs["g2_b2"], f).reshape(1, -1)),
        "wihT1": _bf(T(inputs["r1_Wih"])),
        "wihT2": _bf(T(inputs["r2_Wih"])),
        "whhT1": _bf(T(inputs["r1_Whh"])),
        "whhT2": _bf(T(inputs["r2_Whh"])),
        "w1wT": _bf(T(inputs["w1_W"])),
        "w2wT": _bf(T(inputs["w2_W"])),
        "wb": np.asarray(inputs["w1_b"] + inputs["w2_b"], f).reshape(-1, 1),
        "fccwT": _bf(T(inputs["fcc_W"])),
        "fctwT": _bf(T(inputs["fct_W"])),
        "fcewT": _bf(T(inputs["fce_W"])),
    }
    for u, (ih, hh) in enumerate((("r1_bih", "r1_bhh"), ("r2_bih", "r2_bhh"))):
        bih = np.asarray(inputs[ih], f)
        bhh = np.asarray(inputs[hh], f)
        pb = np.zeros((EMB, 3), f)
        for g in range(3):
            pb[:, g] = bih[g * H:(g + 1) * H]
            if g < 2:  # r, z: fold bhh into the projection bias
                pb[:, g] += bhh[g * H:(g + 1) * H]
        shared[f"projb{u + 1}"] = pb
        shared[f"bhhn{u + 1}"] = bhh[2 * H:3 * H].reshape(-1, 1).copy()

    # transpose each A once, slice + pack per core
    atT = {}
    for g, arr in graphs.items():
        atT[g] = np.ascontiguousarray(np.asarray(arr, f).T)

    in_maps = []
    for c in range(NCORES):
        m = dict(shared)
        for g in graphs:
            atc = np.zeros((NQ, SHARD_P), f)
            atc[:, :SHARD] = atT[g][:, c * SHARD:(c + 1) * SHARD]
            m[f"at_{g}"] = _pack_k(atc)
        xc = x[c * BLOC:(c + 1) * BLOC]           # [8, 200, 5000]
        xts = xc.transpose(2, 1, 0).reshape(NQ, BLC)  # col = t*8 + b
        m["xt"] = np.ascontiguousarray(
            xts.reshape(NK, KC, BLC).transpose(1, 0, 2)
            .reshape(KC, NK * BLC)).astype(ml_dtypes.bfloat16)
        in_maps.append(m)
    return in_maps


def kernel(**inputs):
    global _BUILT, LAST
    from concourse import bass_utils
    if _BUILT is None:
        _BUILT = _build(debug=False)
    nc = _BUILT
    in_maps = _host_prep(inputs)
    res = bass_utils.run_bass_kernel_spmd(nc, in_maps,
                                          core_ids=list(range(NCORES)))
    LAST = res
    f = np.float32
    logit_c = np.empty((B, L, Q), f)
    logit_t = np.empty((B, L, Q), f)
    logit_e = np.empty((B, L, Q), f)
    for c in range(NCORES):
        r = res.results[c]
        logit_c[c * BLOC:(c + 1) * BLOC] = \
            np.asarray(r["out_c"], dtype=f).transpose(1, 0, 2)
        logit_t[c * BLOC:(c + 1) * BLOC] = \
            np.asarray(r["out_t"], dtype=f).transpose(1, 0, 2)
        logit_e[c * BLOC:(c + 1) * BLOC] = \
            np.asarray(r["out_e"], dtype=f).transpose(1, 0, 2)
    for arr, bname in ((logit_c, "fcc_b"), (logit_t, "fct_b"),
                       (logit_e, "fce_b")):
        bias = np.asarray(inputs[bname], f)
        if np.any(bias):
            arr += bias
    return (logit_c, logit_t, logit_e)
